# revision 1
# baseline (speedup 1.0000x reference)
"""Diagonal SSM kernel (Vandermonde contraction) on 8 Trainium2 NeuronCores.

Math: K[d,h,l] = 2*Re( sum_n sc[d,h,n] * w[h,n]^l ),  l in [0, 2048)
  where w = exp(a*dt), sc = c * (exp(a*dt)-1)/a.

Sharding: d_model (H=1024) split contiguously, 128 channels per core.

Strategy (per core): split l = 512*c + j. The host precomputes, in float64,
fp16 basis tables and weights so the device needs NO transcendentals:
  JT[gen, row, (q, j)] : per h-pair rows (h2, n, t): t=0 -> Re(w^j),
                         t=1 -> Im(w^j), j < 512, with decay folded in
  WT[row, (p, c, m)]   : m = (h2, d): block-diagonal weights
                         {2*Re, -2*Im}(sigma), sigma = sc * w^(512c),
                         i.e. the coarse block rotation/decay folded in
Device inner loop, per generation g (4 pairs): one 512 KB JT DMA, then
16 matmuls (k=128, m=4, FD=512) issued q-innermost so the four PSUM
column-groups run concurrently on the PE's 32-column sub-arrays; evacuate
PSUM -> SBUF (alternating ScalarE/VectorE) as fp16 and DMA out full-width.
The host gathers the 16 valid rows per generation and casts to f32.
"""
from contextlib import ExitStack

import numpy as np

import concourse.bass as bass
import concourse.bacc as bacc
import concourse.tile as tile
from concourse import mybir
from concourse.bass_utils import run_bass_kernel_spmd

N_CORES = 8
H = 1024          # d_model
N = 32            # d_state//2
D = 2             # directions
L = 2048          # sequence length
J = 128           # j-block (quarter of a PSUM bank of fp32)
CBLK = L // J     # 16 coarse blocks
HC = H // N_CORES     # 128 channels per core
NPAIR = HC // 2       # 64 pairs per core
NGEN = NPAIR // 4     # 16 generations (4 pairs each)

_nc_cache = {}


def _build_nc(repeat: int = 1, sim_safe: bool = False):
    """Build the Bass program. `repeat` re-runs the whole compute for timing.

    sim_safe=True adds a per-generation PSUM memset so CoreSim's
    initialization tracking accepts the full-tile evacuation reads. The HW
    build skips it (junk PSUM rows are discarded by the host gather) because
    the memset serializes PE behind DVE every generation.
    """
    key = (repeat, sim_safe)
    if key in _nc_cache:
        return _nc_cache[key]
    nc = bacc.Bacc("TRN2", target_bir_lowering=False, debug=False,
                   num_devices=N_CORES)
    f16 = mybir.dt.float16
    f32 = mybir.dt.float32

    # one contiguous [128, 4*J] table per generation (4 pairs side by side)
    jt_d = nc.dram_tensor("jt", [NGEN, 128, 4 * J], f16, kind="ExternalInput")
    wt_d = nc.dram_tensor("wt", [128, NPAIR * CBLK * 4], f16, kind="ExternalInput")
    # device-native layout; host gathers valid rows and casts back to f32.
    # Valid rows are {32q + m, m<4}; the highest is 99, so ship rows 0:100.
    out_d = nc.dram_tensor("out", [NGEN, 100, L], f16, kind="ExternalOutput")

    with tile.TileContext(nc) as tc:
        with ExitStack() as ctx:
            wt_pool = ctx.enter_context(tc.tile_pool(name="wt", bufs=1))
            jt_pool = ctx.enter_context(tc.tile_pool(name="jt", bufs=6))
            st_pool = ctx.enter_context(tc.tile_pool(name="st", bufs=3))
            ps_pool = ctx.enter_context(
                tc.tile_pool(name="ps", bufs=2, space="PSUM"))

            wt = wt_pool.tile([128, NPAIR * CBLK * 4], f16)
            nc.sync.dma_start(wt[:], wt_d.ap())

            for _ in range(repeat):
                for g in range(NGEN):
                    ps = ps_pool.tile([128, L], f32)
                    if sim_safe:
                        nc.vector.memset(ps[:], 0.0)
                    jt = jt_pool.tile([128, 4 * J], f16, tag="jt")
                    # split the 512KB load across both HWDGE trigger engines
                    nc.sync.dma_start(jt[:, :2 * J], jt_d.ap()[g, :, :2 * J])
                    nc.scalar.dma_start(jt[:, 2 * J:], jt_d.ap()[g, :, 2 * J:])
                    # q-innermost: consecutive matmuls hit different PE
                    # col-groups and run concurrently on the 32-col subarrays.
                    # Four 128-wide blocks share one PSUM bank: only the first
                    # writer may clear the bank's has_written bits (start=True);
                    # later ones overwrite their (cleared-bit) quarter.
                    bank_blocks = 512 // J
                    for c in range(CBLK):
                        for q in range(4):
                            p = g * 4 + q
                            wcol = (p * CBLK + c) * 4
                            nc.tensor.matmul(
                                ps[32 * q:32 * q + 4, c * J:(c + 1) * J],
                                wt[:, wcol:wcol + 4],
                                jt[:, q * J:(q + 1) * J],
                                start=(c % bank_blocks == 0),
                                stop=(c % bank_blocks == bank_blocks - 1),
                                tile_position=(0, 32 * q),
                                skip_group_check=True,
                            )
                    st = st_pool.tile([128, L], f16)
                    # evac on alternating engines; both are otherwise idle
                    if g % 2 == 1:
                        nc.scalar.copy(st[:], ps[:])
                    else:
                        nc.vector.tensor_copy(st[:], ps[:])
                    nc.sync.dma_start(out_d.ap()[g], st[0:100])
    nc.compile()
    _nc_cache[key] = nc
    return nc


def _host_tables(log_dt, log_a_real, a_imag, coeffs):
    """Per-core JT/WT tables in float64 -> fp16."""
    dt = np.exp(log_dt.astype(np.float64))                       # [H]
    a = -np.exp(log_a_real.astype(np.float64)) + 1j * a_imag.astype(np.float64)
    da = a * dt[:, None]                                         # [H,N] c128
    w = np.exp(da)
    c = coeffs[..., 0].astype(np.float64) + 1j * coeffs[..., 1].astype(np.float64)
    sc = c * (np.expm1(da) / a)[None]                            # [D,H,N]

    j = np.arange(J, dtype=np.float64)
    # Wj[h,n,j] = w^j : split into decay * phase computed in f64
    re = da.real[:, :, None] * j                                  # [H,N,J]
    im = da.imag[:, :, None] * j
    dec = np.exp(re)
    WjR = dec * np.cos(im)
    WjI = dec * np.sin(im)

    cs = np.arange(CBLK, dtype=np.float64)
    # sigma[d,h,n,c] = sc * w^(J*c)
    wJc = np.exp(da[:, :, None] * (J * cs))                       # [H,N,C]
    sig = sc[:, :, :, None] * wJc[None]                           # [D,H,N,C]

    jts, wts = [], []
    for core in range(N_CORES):
        h0 = core * HC
        # JT[p, 64*h2 + 2*n + t, j]
        jt = np.empty((NPAIR, 2, N, 2, J), np.float64)
        blk_R = WjR[h0:h0 + HC].reshape(NPAIR, 2, N, J)
        blk_I = WjI[h0:h0 + HC].reshape(NPAIR, 2, N, J)
        jt[:, :, :, 0, :] = blk_R
        jt[:, :, :, 1, :] = blk_I
        jt = jt.reshape(NGEN, 4, 128, J).transpose(0, 2, 1, 3)
        jts.append(np.ascontiguousarray(jt.reshape(NGEN, 128, 4 * J),
                                        dtype=np.float16))

        # WT[64*h2p + 2*n + t, (p, c, 2*h2 + d)]
        wt = np.zeros((2, N, 2, NPAIR, CBLK, 2, D), np.float64)
        s = sig[:, h0:h0 + HC].reshape(D, NPAIR, 2, N, CBLK)      # [D,p,h2,n,c]
        for h2 in range(2):
            wt[h2, :, 0, :, :, h2, :] = 2.0 * np.transpose(
                s.real[:, :, h2, :, :], (2, 1, 3, 0))            # [n,p,c,d]
            wt[h2, :, 1, :, :, h2, :] = -2.0 * np.transpose(
                s.imag[:, :, h2, :, :], (2, 1, 3, 0))
        wts.append(wt.reshape(128, NPAIR * CBLK * 4).astype(np.float16))
    return jts, wts


def _gather(results):
    """Assemble [D, H, L] f32 from per-core device-native outs."""
    outs = []
    for c in range(N_CORES):
        o = results[c]["out"]
        if o.shape == (D, HC, L):          # emulate() path
            outs.append(o)
            continue
        # [g, 32*q + m, l]: m = 2*h2 + d -> [d, (g, q, h2), l]
        idx = np.array([32 * q + m for q in range(4) for m in range(4)])
        o = o[:, idx].astype(np.float32)
        o = o.reshape(NGEN, 4, 2, D, L).transpose(3, 0, 1, 2, 4)
        outs.append(o.reshape(D, HC, L))
    return np.concatenate(outs, axis=1)


def kernel(log_dt, log_a_real, a_imag, coeffs, sequence_length, _repeat=1,
           _run=None):
    assert int(sequence_length) == L
    log_dt = np.asarray(log_dt)
    log_a_real = np.asarray(log_a_real)
    a_imag = np.asarray(a_imag)
    coeffs = np.asarray(coeffs)
    jts, wts = _host_tables(log_dt, log_a_real, a_imag, coeffs)
    nc = _build_nc(_repeat)
    in_maps = [{"jt": jts[c], "wt": wts[c]} for c in range(N_CORES)]
    run = _run or (lambda n, m: run_bass_kernel_spmd(
        n, m, core_ids=list(range(N_CORES)), trace=False).results)
    results = run(nc, in_maps)
    return _gather(results)


def emulate(log_dt, log_a_real, a_imag, coeffs, sequence_length):
    """Numpy emulation of the device program (fp16 tables, fp32 accum)."""
    assert int(sequence_length) == L
    jts, wts = _host_tables(log_dt, log_a_real, a_imag, coeffs)
    results = []
    for core in range(N_CORES):
        jt = jts[core].astype(np.float32).reshape(NGEN, 128, 4, J)
        jt = jt.transpose(0, 2, 1, 3).reshape(NPAIR, 128, J)     # [P,128,J]
        wt = wts[core].astype(np.float32).reshape(128, NPAIR, CBLK, 4)
        out = np.empty((D, HC, L), np.float32)
        for p in range(NPAIR):
            for c in range(CBLK):
                # psum[m, j] = sum_k wt[k, p, c, m] * jt[p, k, j]
                pm = wt[:, p, c, :].T @ jt[p]                     # [4, J]
                for h2 in range(2):
                    for d in range(D):
                        out[d, 2 * p + h2, c * J:(c + 1) * J] = pm[2 * h2 + d]
        results.append({"out": out})
    return _gather(results)



# revision 2
# speedup vs baseline: 1.7317x; 1.7317x over previous
"""Diagonal SSM kernel (Vandermonde contraction) on 8 Trainium2 NeuronCores.

Math: K[d,h,l] = 2*Re( sum_n sc[d,h,n] * w[h,n]^l ),  l in [0, 2048)
  where w = exp(a*dt), sc = c * (exp(a*dt)-1)/a.

Sharding: d_model (H=1024) split contiguously, 128 channels per core.

Strategy (per core): split l = J*c + j with J=64, c < 32. Host precomputes
(float64) two fp16 tables so the device does NO transcendentals:
  JT[64*h2 + 2n + t, 64p + j] = {Re,Im}(w[2p+h2, n]^j)        (basis)
  WT[64*h2 + 2n + t, 64p + 32d + c] = {2Re,-2Im}(sc * w^(64c)) (weights)
Device: per pair p two matmuls [k=64, m=64=(d,c), n=64=j] on diagonal PE
quadrants write PSUM partitions (h2, d, c) x columns j; 8 pairs fill one
PSUM bank [128, 512]f32; one copy per bank evacuates to a dense
[128, 4096]f16 staging tile; 4 wide DMAs ship it out. All DMAs move
>=2KB per partition line at full DMA-engine rate, and total HBM traffic
is 3 MB/core (vs 9.6 MB for the naive row-padded layout).
"""
from contextlib import ExitStack

import numpy as np

import concourse.bass as bass
import concourse.bacc as bacc
import concourse.tile as tile
from concourse import mybir
from concourse.bass_utils import run_bass_kernel_spmd

N_CORES = 8
H = 1024          # d_model
N = 32            # d_state//2
D = 2             # directions
L = 2048          # sequence length
J = 64            # j-block
CBLK = L // J     # 32 coarse blocks
HC = H // N_CORES     # 128 channels per core
NPAIR = HC // 2       # 64 pairs per core
NCHUNK = 4            # jt load chunks
PPC = NPAIR // NCHUNK     # 16 pairs per chunk
NBANK = 8                 # PSUM banks
PPB = NPAIR // NBANK      # 8 pairs per bank

JT_DT = "float16"         # basis table dtype on device
WT_DT = "float16"         # weight table dtype on device

_nc_cache = {}


def _build_nc(repeat: int = 1, jt_dt: str = None, wt_dt: str = None):
    """Build the Bass program. `repeat` re-runs the whole compute for timing."""
    jt_dt = jt_dt or JT_DT
    wt_dt = wt_dt or WT_DT
    key = (repeat, jt_dt, wt_dt)
    if key in _nc_cache:
        return _nc_cache[key]
    nc = bacc.Bacc("TRN2", target_bir_lowering=False, debug=False,
                   num_devices=N_CORES)
    f16 = mybir.dt.float16
    f32 = mybir.dt.float32
    djt = getattr(mybir.dt, jt_dt)
    dwt = getattr(mybir.dt, wt_dt)

    jt_d = nc.dram_tensor("jt", [128, NPAIR * J], djt, kind="ExternalInput")
    wt_d = nc.dram_tensor("wt", [128, NPAIR * 64], dwt, kind="ExternalInput")
    out_d = nc.dram_tensor("out", [128, NPAIR * J], f16, kind="ExternalOutput")

    with tile.TileContext(nc) as tc:
        with ExitStack() as ctx:
            wt_pool = ctx.enter_context(tc.tile_pool(name="wt", bufs=2))
            jt_pool = ctx.enter_context(tc.tile_pool(name="jt", bufs=NCHUNK + 1))
            ob_pool = ctx.enter_context(tc.tile_pool(name="ob", bufs=2))
            ps_pool = ctx.enter_context(
                tc.tile_pool(name="ps", bufs=NBANK, space="PSUM"))

            for _ in range(repeat):
                wt = wt_pool.tile([128, NPAIR * 64], dwt, tag="wt")
                nc.sync.dma_start(wt[:, :NPAIR * 32], wt_d.ap()[:, :NPAIR * 32])
                nc.sync.dma_start(wt[:, NPAIR * 32:], wt_d.ap()[:, NPAIR * 32:])
                jts = []
                for ch in range(NCHUNK):
                    t = jt_pool.tile([128, PPC * J], djt, tag="jt")
                    nc.scalar.dma_start(
                        t[:], jt_d.ap()[:, ch * PPC * J:(ch + 1) * PPC * J])
                    jts.append(t)
                ob = ob_pool.tile([128, NPAIR * J], f16, tag="ob")

                ps = None
                for p in range(NPAIR):
                    b, col = p // PPB, (p % PPB) * J
                    if p % PPB == 0:
                        ps = ps_pool.tile([128, PPB * J], f32)
                    jt = jts[p // PPC]
                    pc = (p % PPC) * J
                    for h2 in (0, 1):
                        nc.tensor.matmul(
                            ps[64 * h2:64 * h2 + 64, col:col + J],
                            wt[64 * h2:64 * h2 + 64, 64 * p:64 * p + 64],
                            jt[64 * h2:64 * h2 + 64, pc:pc + J],
                            start=True, stop=True,
                            tile_position=(64 * h2, 64 * h2),
                            skip_group_check=True,
                        )
                    if p % PPB == PPB - 1:
                        sl = slice(b * PPB * J, (b + 1) * PPB * J)
                        if b % 2 == 1:
                            nc.scalar.copy(ob[:, sl], ps[:])
                        else:
                            nc.vector.tensor_copy(ob[:, sl], ps[:])
                        if b % 2 == 1:
                            osl = slice((b - 1) * PPB * J, (b + 1) * PPB * J)
                            nc.sync.dma_start(out_d.ap()[:, osl], ob[:, osl])
    nc.compile()
    _nc_cache[key] = nc
    return nc


def _host_tables(log_dt, log_a_real, a_imag, coeffs, jt_dt: str = None,
                 wt_dt: str = None):
    """Per-core JT/WT tables in float64 -> device dtypes."""
    np_jt = mybir.dt.np(getattr(mybir.dt, jt_dt or JT_DT))
    np_wt = mybir.dt.np(getattr(mybir.dt, wt_dt or WT_DT))
    dt = np.exp(log_dt.astype(np.float64))                       # [H]
    a = -np.exp(log_a_real.astype(np.float64)) + 1j * a_imag.astype(np.float64)
    da = a * dt[:, None]                                         # [H,N] c128
    c = coeffs[..., 0].astype(np.float64) + 1j * coeffs[..., 1].astype(np.float64)
    sc = c * (np.expm1(da) / a)[None]                            # [D,H,N]

    j = np.arange(J, dtype=np.float64)
    WjR = np.exp(da.real[:, :, None] * j) * np.cos(da.imag[:, :, None] * j)
    WjI = np.exp(da.real[:, :, None] * j) * np.sin(da.imag[:, :, None] * j)

    cs = np.arange(CBLK, dtype=np.float64)
    wJc = np.exp(da[:, :, None] * (J * cs))                      # [H,N,C]
    sig = sc[:, :, :, None] * wJc[None]                          # [D,H,N,C]

    jts, wts = [], []
    for core in range(N_CORES):
        h0 = core * HC
        # JT rows (h2, n, t), cols (p, j)
        jt = np.empty((2, N, 2, NPAIR, J), np.float64)
        R = WjR[h0:h0 + HC].reshape(NPAIR, 2, N, J).transpose(1, 2, 0, 3)
        I = WjI[h0:h0 + HC].reshape(NPAIR, 2, N, J).transpose(1, 2, 0, 3)
        jt[:, :, 0] = R
        jt[:, :, 1] = I
        jts.append(jt.reshape(128, NPAIR * J).astype(np_jt))

        # WT rows (h2, n, t), cols (p, d, c)
        s = sig[:, h0:h0 + HC].reshape(D, NPAIR, 2, N, CBLK)     # [d,p,h2,n,c]
        wt = np.empty((2, N, 2, NPAIR, D, CBLK), np.float64)
        wt[:, :, 0] = 2.0 * s.real.transpose(2, 3, 1, 0, 4)      # [h2,n,p,d,c]
        wt[:, :, 1] = -2.0 * s.imag.transpose(2, 3, 1, 0, 4)
        wts.append(wt.reshape(128, NPAIR * 64).astype(np_wt))
    return jts, wts


def _gather(results):
    """Assemble [D, H, L] f32 from per-core outs [128, NPAIR*J] f16."""
    outs = []
    for c in range(N_CORES):
        o = np.asarray(results[c]["out"]).astype(np.float32)
        # rows (h2, d, c), cols (p, j) -> [d, (p, h2), (c, j)]
        arr = o.reshape(2, D, CBLK, NPAIR, J).transpose(1, 3, 0, 2, 4)
        outs.append(arr.reshape(D, HC, L))
    return np.concatenate(outs, axis=1)


def kernel(log_dt, log_a_real, a_imag, coeffs, sequence_length, _repeat=1,
           _run=None):
    assert int(sequence_length) == L
    log_dt = np.asarray(log_dt)
    log_a_real = np.asarray(log_a_real)
    a_imag = np.asarray(a_imag)
    coeffs = np.asarray(coeffs)
    jts, wts = _host_tables(log_dt, log_a_real, a_imag, coeffs)
    nc = _build_nc(_repeat)
    in_maps = [{"jt": jts[c], "wt": wts[c]} for c in range(N_CORES)]
    run = _run or (lambda n, m: run_bass_kernel_spmd(
        n, m, core_ids=list(range(N_CORES)), trace=False).results)
    results = run(nc, in_maps)
    return _gather(results)


def emulate(log_dt, log_a_real, a_imag, coeffs, sequence_length):
    """Numpy emulation of the device program (quantized tables, f32 accum)."""
    assert int(sequence_length) == L
    jts, wts = _host_tables(log_dt, log_a_real, a_imag, coeffs)
    results = []
    for core in range(N_CORES):
        jt = jts[core].astype(np.float32)
        wt = wts[core].astype(np.float32)
        out = np.zeros((128, NPAIR * J), np.float32)
        for p in range(NPAIR):
            for h2 in (0, 1):
                blk = wt[64 * h2:64 * h2 + 64, 64 * p:64 * p + 64].T \
                    @ jt[64 * h2:64 * h2 + 64, p * J:(p + 1) * J]
                out[64 * h2:64 * h2 + 64, p * J:(p + 1) * J] = blk
        results.append({"out": out.astype(np.float16)})
    return _gather(results)
